# revision 3
# baseline (speedup 1.0000x reference)
"""MiMoV2 MoE gate (moe_routing) on 8 Trainium2 NeuronCores — v3.

Strategy ("detect and repair"):
  - Shard tokens (16384) across 8 cores, 2048 each; replicate gate weight.
  - MAIN pass: single fp16 gating GEMM (x1*W1) + LUT sigmoid + full routing.
    fp16 logits have ~5e-4 rms error -> ~1% of tokens pick wrong experts.
  - DETECT: per token, measure the minimum ranking-boundary gap
    (adjacent top-8 gaps, 8th-vs-9th, group 4-vs-5). Tokens with any gap
    < THETA are flagged (the only candidates for expert flips).
  - REPAIR on-device: compact flagged token ids (gpsimd sparse_gather),
    dma_gather their fp16 hi/lo columns from token-major HBM copies, and
    recompute exact (3-pass hi/lo) logits + routing for <=128 tokens per
    1024-token wave. Host overwrites flagged rows during unsharding.
  - Device outputs top-8 indices + top-8 biased scores; host does the
    trivial O(T*8) tail: raw = v8 - bias[idx]; w = raw/sum(raw)*2.5.

Inputs (full):  hidden_states [4,4096,4096] f32, weight [256,4096] f32,
                e_score_correction_bias [256] f32
Output (full):  (topk_idx [16384,8] int32, topk_weight [16384,8] f32)
"""

import numpy as np

import concourse.tile as tile
from concourse import bacc, mybir
from concourse.bass_utils import run_bass_kernel_spmd

# problem shape (hardcoded per contract)
T_FULL = 16384
H = 4096
E = 256
G = 8
GS = E // G           # 32
TOPK = 8
SCALING = 2.5

N_CORES = 8
T_CORE = T_FULL // N_CORES    # 2048
ST_SIZES = [128, 128] + [256] * 6 + [128, 128]
assert sum(ST_SIZES) == T_CORE
ST_OFFS = [sum(ST_SIZES[:i]) for i in range(len(ST_SIZES))]
ST_MAX = max(ST_SIZES)
N_CHUNK = H // 128            # 32 contraction chunks
HC = N_CHUNK // 2             # 16
QC = HC // 2                  # 8 chunks per quarter-tile

CROSS = 4096.0                # 2^12 residual prescale (dodges fp16 subnormals)
THETA = 1.1e-4                # boundary-gap flag threshold
NSUB = T_CORE // 128          # 16 subtiles
WAVES = 2
SUB_W = NSUB // WAVES         # 8 subtiles per wave
CAP = 128                     # repair capacity per wave (tokens)

_BUILT = None


def _build(trace=False):
    f32 = mybir.dt.float32
    f16 = mybir.dt.float16
    u16 = mybir.dt.uint16
    u32 = mybir.dt.uint32
    i16 = mybir.dt.int16
    AF = mybir.ActivationFunctionType
    OP = mybir.AluOpType
    AX = mybir.AxisListType

    nc = bacc.Bacc("TRN2", target_bir_lowering=False, debug=False)

    x1 = nc.dram_tensor("x1", [128, N_CHUNK * T_CORE], f16, kind="ExternalInput").ap()
    w1 = nc.dram_tensor("w1", [2, 128, HC * E], f16, kind="ExternalInput").ap()
    w2 = nc.dram_tensor("w2", [2, 128, HC * E], f16, kind="ExternalInput").ap()
    bias_rep = nc.dram_tensor("bias_rep", [128, E], f32, kind="ExternalInput").ap()
    x1t = nc.dram_tensor("x1t", [T_CORE, H], f16, kind="ExternalInput").ap()
    x2t = nc.dram_tensor("x2t", [T_CORE, H], f16, kind="ExternalInput").ap()
    tid1 = nc.dram_tensor("tid1", [128, NSUB], f32, kind="ExternalInput").ap()
    ident = nc.dram_tensor("ident", [128, 128], f16, kind="ExternalInput").ap()
    repmat = nc.dram_tensor("repmat", [16, 128], f16, kind="ExternalInput").ap()

    idx_out = nc.dram_tensor("idx_out", [T_CORE, TOPK], u16, kind="ExternalOutput").ap()
    vch_out = nc.dram_tensor("vch_out", [T_CORE, TOPK], f32, kind="ExternalOutput").ap()
    rep_i = nc.dram_tensor("rep_i", [WAVES, CAP, TOPK], u16, kind="ExternalOutput").ap()
    rep_v = nc.dram_tensor("rep_v", [WAVES, CAP, TOPK], f32, kind="ExternalOutput").ap()
    rep_ids = nc.dram_tensor("rep_ids", [WAVES, 16, CAP // 16], f32,
                             kind="ExternalOutput").ap()
    rep_n = nc.dram_tensor("rep_n", [WAVES, 1], u32, kind="ExternalOutput").ap()

    w1v = w1.rearrange("h p (c e) -> h p c e", c=HC)
    w2v = w2.rearrange("h p (c e) -> h p c e", c=HC)

    with tile.TileContext(nc) as tc:
        with tc.tile_pool(name="const", bufs=1) as cpool, \
             tc.tile_pool(name="xin", bufs=3) as xpool, \
             tc.tile_pool(name="mid", bufs=4) as mpool, \
             tc.tile_pool(name="rmid", bufs=1) as rmpool, \
             tc.tile_pool(name="small", bufs=8) as spool, \
             tc.tile_pool(name="wv", bufs=2) as wpool, \
             tc.tile_pool(name="gat", bufs=2) as gpool, \
             tc.tile_pool(name="psum", bufs=3, space="PSUM") as ppool, \
             tc.tile_pool(name="psrep", bufs=1, space="PSUM") as prpool, \
             tc.tile_pool(name="pst", bufs=1, space="PSUM") as ptpool, \
             tc.tile_pool(name="psid", bufs=1, space="PSUM") as pipool:

            # --- constants: W quarters first (matmuls wait on these) ---
            Wt_ = {}
            for nm in ("W1", "W2"):
                for h in range(2):
                    for q in range(2):
                        Wt_[nm, 2 * h + q] = cpool.tile([128, QC, E], f16,
                                                        name=f"{nm}{h}{q}",
                                                        tag=f"{nm}{h}{q}")
            BR = cpool.tile([128, E], f32, tag="BR")
            TID = cpool.tile([128, NSUB], f32, tag="TID")
            IDE = cpool.tile([128, 128], f16, tag="IDE")
            RPM = cpool.tile([16, 128], f16, tag="RPM")
            for q in range(4):
                nc.sync.dma_start(Wt_["W1", q][:],
                                  w1v[q // 2][:, (q % 2) * QC:(q % 2 + 1) * QC, :])
                nc.sync.dma_start(Wt_["W2", q][:],
                                  w2v[q // 2][:, (q % 2) * QC:(q % 2 + 1) * QC, :])
                if q == 0:
                    nc.sync.dma_start(BR[:], bias_rep)
                    nc.sync.dma_start(TID[:], tid1)
                    nc.sync.dma_start(IDE[:], ident)
                    nc.sync.dma_start(RPM[:], repmat)

            def Wc(nm, c):
                return Wt_[nm, c // QC][:, c % QC, :]

            # routing on a [128,256] f32 score tile; returns (v8, i8)
            def route(s_raw, pref, mpool):
                s_choice = mpool.tile([128, E], f32, tag=f"{pref}sc")
                nc.gpsimd.tensor_tensor(s_choice[:], s_raw[:], BR[:], op=OP.add)
                sc3 = s_choice[:].rearrange("p (g s) -> p g s", g=G)
                m1 = spool.tile([128, G], f32, tag=f"{pref}m1")
                nc.vector.reduce_max(m1[:], sc3, axis=AX.X)
                repl = mpool.tile([128, E], f32, tag=f"{pref}rp")
                nc.vector.match_replace(repl[:], m1[:], s_choice[:], -1e30)
                m2 = spool.tile([128, G], f32, tag=f"{pref}m2")
                nc.vector.reduce_max(m2[:], repl[:].rearrange("p (g s) -> p g s", g=G),
                                     axis=AX.X)
                gsum = spool.tile([128, G], f32, tag=f"{pref}gs")
                nc.gpsimd.tensor_tensor(gsum[:], m1[:], m2[:], op=OP.add)
                gs8 = spool.tile([128, 8], f32, tag=f"{pref}g8")
                nc.vector.max(gs8[:], gsum[:])
                pen = spool.tile([128, G], f32, tag=f"{pref}pn")
                nc.gpsimd.tensor_scalar(pen[:], gsum[:], gs8[:, 3:4], -1e30,
                                        op0=OP.is_lt, op1=OP.mult)
                s_mask = mpool.tile([128, E], f32, tag=f"{pref}sm")
                pen_b = pen[:].unsqueeze(2).broadcast_to([128, G, GS])
                nc.gpsimd.tensor_tensor(
                    s_mask[:].rearrange("p (g s) -> p g s", g=G), sc3, pen_b,
                    op=OP.add)
                v8 = spool.tile([128, 8], f32, tag=f"{pref}v8")
                nc.vector.max(v8[:], s_mask[:])
                i8 = spool.tile([128, 8], u16, tag=f"{pref}i8")
                nc.vector.max_index(i8[:], v8[:], s_mask[:])
                return s_mask, gs8, v8, i8

            fvw = {w: wpool.tile([128, 16], f16, name=f"fv{w}", tag=f"fv{w}")
                   for w in range(WAVES)}

            def wave_tail(w):
                # pad cols 8..16 with -1 (never-flagged filler)
                nc.scalar.activation(fvw[w][:, 8:16], TID[:, 0:8], AF.Copy,
                                     scale=0.0, bias=-1.0)
                pt = ptpool.tile([16, 128], f16, tag="pt")
                nc.tensor.transpose(pt[:], fvw[w][:], IDE[:])
                fv32 = wpool.tile([16, 128], f32, tag="fv32")
                nc.scalar.activation(fv32[:], pt[:], AF.Copy)
                svals = wpool.tile([16, CAP // 16], f32, tag="svals")
                nf = wpool.tile([1, 1], u32, tag="nf")
                nc.gpsimd.sparse_gather(svals[:], fv32[:], num_found=nf[:])
                nc.sync.dma_start(rep_ids[w], svals[:])
                nc.sync.dma_start(rep_n[w:w + 1, :], nf[:])
                svc = wpool.tile([16, CAP // 16], f32, tag="svc")
                nc.vector.tensor_scalar(svc[:], svals[:], 0.0, float(T_CORE - 1),
                                        op0=OP.max, op1=OP.min)
                svh = wpool.tile([16, CAP // 16], f16, tag="svh")
                nc.vector.tensor_scalar(svh[:], svc[:], 0.0, None, op0=OP.add)
                ip = pipool.tile([128, CAP // 16], f32, tag="ip")
                nc.tensor.matmul(ip[:], RPM[:], svh[:], start=True, stop=True)
                ids16 = wpool.tile([128, CAP // 16], i16, tag="ids16")
                nc.vector.tensor_scalar(ids16[:], ip[:], 0.0, None, op0=OP.add)
                x1g = gpool.tile([128, N_CHUNK, CAP], f16, tag="x1g")
                x2g = gpool.tile([128, N_CHUNK, CAP], f16, tag="x2g")
                nc.gpsimd.dma_gather(x1g[:], x1t, ids16[:], CAP, CAP, H,
                                     transpose=True)
                nc.gpsimd.dma_gather(x2g[:], x2t, ids16[:], CAP, CAP, H,
                                     transpose=True)

                pr = prpool.tile([128, E], f32, tag="pr")
                pr2 = prpool.tile([128, E], f32, tag="pr2")
                for c in range(N_CHUNK):
                    nc.tensor.matmul(pr[:], x1g[:, c, :], Wc("W1", c),
                                     start=(c == 0), stop=(c == N_CHUNK - 1))
                for c in range(N_CHUNK):
                    nc.tensor.matmul(pr2[:], x1g[:, c, :], Wc("W2", c),
                                     start=(c == 0), stop=False)
                    nc.tensor.matmul(pr2[:], x2g[:, c, :], Wc("W1", c),
                                     start=False, stop=(c == N_CHUNK - 1))
                # exact sigmoid: 1/(1+exp(-x)), x = pr + pr2/CROSS
                t2n = rmpool.tile([128, E], f32, tag="rt2")
                nc.scalar.activation(t2n[:], pr2[:], AF.Copy, scale=-1.0 / CROSS)
                lgn = rmpool.tile([128, E], f32, tag="rlg")
                nc.vector.scalar_tensor_tensor(lgn[:], in0=pr[:], scalar=-1.0,
                                               in1=t2n[:], op0=OP.mult, op1=OP.add)
                ex = rmpool.tile([128, E], f32, tag="rex")
                nc.scalar.activation(ex[:], lgn[:], AF.Exp)
                ip1 = rmpool.tile([128, E], f32, tag="rip")
                nc.scalar.activation(ip1[:], ex[:], AF.Copy, bias=1.0)
                s_raw_r = rmpool.tile([128, E], f32, tag="rsr")
                rscr = rmpool.tile([128, E], f32, tag="rrs")
                nc.vector.reciprocal_approx_accurate(s_raw_r[:], ip1[:], rscr[:])
                _, _, rv8, ri8 = route(s_raw_r, "r", rmpool)
                nc.sync.dma_start(rep_v[w], rv8[:])
                nc.sync.dma_start(rep_i[w], ri8[:])

            # --- main loop ---
            for st, (toff, tsz) in enumerate(zip(ST_OFFS, ST_SIZES)):
                xt1 = xpool.tile([128, N_CHUNK, ST_MAX], f16, tag="xt1")
                seg = slice(N_CHUNK * toff, N_CHUNK * (toff + tsz))
                nc.sync.dma_start(xt1[:, :, 0:tsz],
                                  x1[:, seg].rearrange("p (c t) -> p c t", c=N_CHUNK))

                for sub in range(tsz // 128):
                    tok0 = toff + sub * 128
                    s = tok0 // 128          # global subtile 0..15
                    wv = s // SUB_W
                    ps = ppool.tile([128, E], f32, tag="ps")
                    for c in range(N_CHUNK):
                        nc.tensor.matmul(ps[:], xt1[:, c, sub * 128:(sub + 1) * 128],
                                         Wc("W1", c),
                                         start=(c == 0), stop=(c == N_CHUNK - 1))
                    s_raw = mpool.tile([128, E], f32, tag="sr")
                    nc.scalar.activation(s_raw[:], ps[:], AF.Sigmoid)
                    s_mask, gs8, v8, i8 = route(s_raw, "m", mpool)
                    nc.sync.dma_start(idx_out[tok0:tok0 + 128, :], i8[:])
                    nc.sync.dma_start(vch_out[tok0:tok0 + 128, :], v8[:])

                    # --- flag near-boundary tokens ---
                    thr9 = spool.tile([128, 1], f32, tag="t9")
                    nc.scalar.activation(thr9[:], v8[:, 7:8], AF.Copy, bias=-THETA)
                    ge9 = mpool.tile([128, E], f32, tag="ge9")
                    nc.gpsimd.tensor_scalar(ge9[:], s_mask[:], thr9[:, 0:1], None,
                                            op0=OP.is_ge)
                    cnt = spool.tile([128, 1], f32, tag="cnt")
                    nc.vector.tensor_reduce(cnt[:], ge9[:], axis=AX.X, op=OP.add)
                    ggap = spool.tile([128, 1], f32, tag="gg")
                    nc.scalar.activation(ggap[:], gs8[:, 4:5], AF.Identity,
                                         scale=-1.0, bias=gs8[:, 3:4])
                    adj = spool.tile([128, 7], f32, tag="adj")
                    nc.gpsimd.tensor_tensor(adj[:], v8[:, 0:7], v8[:, 1:8],
                                            op=OP.subtract)
                    adjmin = spool.tile([128, 1], f32, tag="am")
                    nc.vector.tensor_reduce(adjmin[:], adj[:], axis=AX.X, op=OP.min)
                    # flag = (adjmin<θ) | (ggap<θ) | (cnt>8.5), via indicator sum
                    f1 = spool.tile([128, 1], f32, tag="f1")
                    nc.gpsimd.tensor_scalar(f1[:], adjmin[:], THETA, None,
                                            op0=OP.is_lt)
                    f2 = spool.tile([128, 1], f32, tag="f2")
                    nc.gpsimd.tensor_scalar(f2[:], ggap[:], THETA, None,
                                            op0=OP.is_lt)
                    f3 = spool.tile([128, 1], f32, tag="f3")
                    nc.gpsimd.tensor_scalar(f3[:], cnt[:], 8.5, None, op0=OP.is_gt)
                    s1 = spool.tile([128, 1], f32, tag="s1")
                    nc.gpsimd.tensor_tensor(s1[:], f1[:], f2[:], op=OP.add)
                    s2 = spool.tile([128, 1], f32, tag="s2")
                    nc.gpsimd.tensor_tensor(s2[:], s1[:], f3[:], op=OP.add)
                    f12 = spool.tile([128, 1], f32, tag="f12")
                    nc.gpsimd.tensor_scalar(f12[:], s2[:], 0.5, None, op0=OP.is_gt)
                    # fv = flag * (tid+1) - 1  (tid if flagged else -1)
                    nc.scalar.activation(fvw[wv][:, (s % SUB_W):(s % SUB_W) + 1],
                                         f12[:], AF.Copy,
                                         scale=TID[:, s:s + 1], bias=-1.0)

                    if s == SUB_W - 1:
                        wave_tail(0)
                    elif s == NSUB - 1:
                        wave_tail(1)

    nc.compile()
    return nc


def _get_built():
    global _BUILT
    if _BUILT is None:
        _BUILT = _build()
    return _BUILT


def _tile_x(arr):
    # [H, T_CORE] -> [128p, N_CHUNK*T_CORE] supertile-major (c-major, token-minor)
    v = arr.reshape(N_CHUNK, 128, T_CORE)
    segs = [np.ascontiguousarray(v[:, :, o:o + s].transpose(1, 0, 2)
                                 ).reshape(128, N_CHUNK * s)
            for o, s in zip(ST_OFFS, ST_SIZES)]
    return np.ascontiguousarray(np.concatenate(segs, axis=1))


def _tile_w(arr):
    # [H, E] -> [2, 128p, HC*E] with (p,c,e): arr[(h*HC+c)*128+p, e]
    v = arr.reshape(2, HC, 128, E)
    return np.ascontiguousarray(v.transpose(0, 2, 1, 3).reshape(2, 128, HC * E))


def _prep_in_maps(hidden_states, weight, e_score_correction_bias):
    S = np.float32(CROSS)
    x = np.asarray(hidden_states, dtype=np.float32).reshape(T_FULL, H)
    x1_tok = x.astype(np.float16)                        # [T, H] token-major
    x2_tok = ((x - x1_tok.astype(np.float32)) * S).astype(np.float16)

    W = np.asarray(weight, dtype=np.float32)
    Wt = np.ascontiguousarray(W.T)                       # [H, E]
    w1 = _tile_w(Wt.astype(np.float16))
    w2 = _tile_w(((Wt - Wt.astype(np.float16).astype(np.float32)) * S
                  ).astype(np.float16))

    b = np.asarray(e_score_correction_bias, dtype=np.float32)
    bias_rep = np.ascontiguousarray(np.tile(b[None, :], (128, 1)))

    tid1 = np.ascontiguousarray(
        (np.arange(T_CORE, dtype=np.float32).reshape(NSUB, 128).T + 1.0))
    ident = np.eye(128, dtype=np.float16)
    repmat = np.ascontiguousarray(np.tile(np.eye(16, dtype=np.float16), (1, 8)))

    in_maps = []
    for c in range(N_CORES):
        sl = slice(c * T_CORE, (c + 1) * T_CORE)
        x1t_c = np.ascontiguousarray(x1_tok[sl])         # [T_CORE, H]
        x2t_c = np.ascontiguousarray(x2_tok[sl])
        in_maps.append({
            "x1": _tile_x(np.ascontiguousarray(x1t_c.T)),
            "w1": w1, "w2": w2, "bias_rep": bias_rep,
            "x1t": x1t_c, "x2t": x2t_c,
            "tid1": tid1, "ident": ident, "repmat": repmat,
        })
    return in_maps


def _merge_core(res, b):
    """Merge one core's outputs -> (idx int32 [T_CORE,8], w f32 [T_CORE,8])."""
    idx = res["idx_out"].astype(np.int32)
    v8 = res["vch_out"].astype(np.float32).copy()
    rep_i = res["rep_i"]
    rep_v = res["rep_v"]
    rep_ids = res["rep_ids"]
    rep_n = res["rep_n"]
    for w in range(WAVES):
        n = min(int(rep_n[w][0]), CAP)
        if n <= 0:
            continue
        j = np.arange(n)
        toks = rep_ids[w][j % 16, j // 16].astype(np.int64)
        idx[toks] = rep_i[w][:n].astype(np.int32)
        v8[toks] = rep_v[w][:n]
    raw = v8 - b[idx]
    wt = raw / (raw.sum(axis=-1, keepdims=True) / np.float32(SCALING))
    return idx, wt.astype(np.float32)


def kernel(hidden_states: np.ndarray, weight: np.ndarray,
           e_score_correction_bias: np.ndarray):
    in_maps = _prep_in_maps(hidden_states, weight, e_score_correction_bias)
    nc = _get_built()
    res = run_bass_kernel_spmd(nc, in_maps, list(range(N_CORES)))
    b = np.asarray(e_score_correction_bias, dtype=np.float32)
    outs = [_merge_core(r, b) for r in res.results]
    idx = np.concatenate([o[0] for o in outs], axis=0)
    w = np.concatenate([o[1] for o in outs], axis=0)
    return idx, w


# revision 4
# speedup vs baseline: 1.4135x; 1.4135x over previous
"""MiMoV2 MoE gate (moe_routing) on 8 Trainium2 NeuronCores — v3.

Strategy ("detect and repair"):
  - Shard tokens (16384) across 8 cores, 2048 each; replicate gate weight.
  - MAIN pass: single fp16 gating GEMM (x1*W1) + LUT sigmoid + full routing.
    fp16 logits have ~5e-4 rms error -> ~1% of tokens pick wrong experts.
  - DETECT: per token, measure the minimum ranking-boundary gap
    (adjacent top-8 gaps, 8th-vs-9th, group 4-vs-5). Tokens with any gap
    < THETA are flagged (the only candidates for expert flips).
  - REPAIR on-device: compact flagged token ids (gpsimd sparse_gather),
    dma_gather their fp16 hi/lo columns from token-major HBM copies, and
    recompute exact (3-pass hi/lo) logits + routing for <=128 tokens per
    1024-token wave. Host overwrites flagged rows during unsharding.
  - Device outputs top-8 indices + top-8 biased scores; host does the
    trivial O(T*8) tail: raw = v8 - bias[idx]; w = raw/sum(raw)*2.5.

Inputs (full):  hidden_states [4,4096,4096] f32, weight [256,4096] f32,
                e_score_correction_bias [256] f32
Output (full):  (topk_idx [16384,8] int32, topk_weight [16384,8] f32)
"""

import numpy as np

import concourse.tile as tile
from concourse import bacc, mybir
from concourse.bass_utils import run_bass_kernel_spmd

# problem shape (hardcoded per contract)
T_FULL = 16384
H = 4096
E = 256
G = 8
GS = E // G           # 32
TOPK = 8
SCALING = 2.5

N_CORES = 8
T_CORE = T_FULL // N_CORES    # 2048
ST_SIZES = [128, 128] + [256] * 6 + [128, 128]
assert sum(ST_SIZES) == T_CORE
ST_OFFS = [sum(ST_SIZES[:i]) for i in range(len(ST_SIZES))]
ST_MAX = max(ST_SIZES)
N_CHUNK = H // 128            # 32 contraction chunks
HC = N_CHUNK // 2             # 16
QC = HC // 2                  # 8 chunks per quarter-tile

CROSS = 4096.0                # 2^12 residual prescale (dodges fp16 subnormals)
THETA = 1.1e-4                # boundary-gap flag threshold
NSUB = T_CORE // 128          # 16 subtiles
WAVES = 2
SUB_W = NSUB // WAVES         # 8 subtiles per wave
CAP = 128                     # repair capacity per wave (tokens)

_BUILT = None


def _build(trace=False):
    f32 = mybir.dt.float32
    f16 = mybir.dt.float16
    u16 = mybir.dt.uint16
    u32 = mybir.dt.uint32
    i16 = mybir.dt.int16
    AF = mybir.ActivationFunctionType
    OP = mybir.AluOpType
    AX = mybir.AxisListType

    nc = bacc.Bacc("TRN2", target_bir_lowering=False, debug=False)

    x1 = nc.dram_tensor("x1", [128, N_CHUNK * T_CORE], f16, kind="ExternalInput").ap()
    w1 = nc.dram_tensor("w1", [2, 128, HC * E], f16, kind="ExternalInput").ap()
    w2 = nc.dram_tensor("w2", [2, 128, HC * E], f16, kind="ExternalInput").ap()
    bias_rep = nc.dram_tensor("bias_rep", [128, E], f32, kind="ExternalInput").ap()
    x1t = nc.dram_tensor("x1t", [T_CORE, H], f16, kind="ExternalInput").ap()
    x2t = nc.dram_tensor("x2t", [T_CORE, H], f16, kind="ExternalInput").ap()
    tid1 = nc.dram_tensor("tid1", [128, NSUB], f32, kind="ExternalInput").ap()
    ident = nc.dram_tensor("ident", [128, 128], f16, kind="ExternalInput").ap()
    repmat = nc.dram_tensor("repmat", [16, 128], f16, kind="ExternalInput").ap()

    idx_out = nc.dram_tensor("idx_out", [T_CORE, TOPK], u16, kind="ExternalOutput").ap()
    vch_out = nc.dram_tensor("vch_out", [T_CORE, TOPK], f32, kind="ExternalOutput").ap()
    rep_i = nc.dram_tensor("rep_i", [WAVES, CAP, TOPK], u16, kind="ExternalOutput").ap()
    rep_v = nc.dram_tensor("rep_v", [WAVES, CAP, TOPK], f32, kind="ExternalOutput").ap()
    rep_ids = nc.dram_tensor("rep_ids", [WAVES, 16, CAP // 16], f32,
                             kind="ExternalOutput").ap()
    rep_n = nc.dram_tensor("rep_n", [WAVES, 1], u32, kind="ExternalOutput").ap()

    w1v = w1.rearrange("h p (c e) -> h p c e", c=HC)
    w2v = w2.rearrange("h p (c e) -> h p c e", c=HC)

    with tile.TileContext(nc) as tc:
        with tc.tile_pool(name="const", bufs=1) as cpool, \
             tc.tile_pool(name="xin", bufs=3) as xpool, \
             tc.tile_pool(name="mid", bufs=4) as mpool, \
             tc.tile_pool(name="rmid", bufs=1) as rmpool, \
             tc.tile_pool(name="small", bufs=8) as spool, \
             tc.tile_pool(name="wv", bufs=2) as wpool, \
             tc.tile_pool(name="gat", bufs=2) as gpool, \
             tc.tile_pool(name="psum", bufs=3, space="PSUM") as ppool, \
             tc.tile_pool(name="psrep", bufs=1, space="PSUM") as prpool, \
             tc.tile_pool(name="pst", bufs=1, space="PSUM") as ptpool, \
             tc.tile_pool(name="psid", bufs=1, space="PSUM") as pipool:

            # --- constants: W quarters first (matmuls wait on these) ---
            Wt_ = {}
            for nm in ("W1", "W2"):
                for h in range(2):
                    for q in range(2):
                        Wt_[nm, 2 * h + q] = cpool.tile([128, QC, E], f16,
                                                        name=f"{nm}{h}{q}",
                                                        tag=f"{nm}{h}{q}")
            BR = cpool.tile([128, E], f32, tag="BR")
            TID = cpool.tile([128, NSUB], f32, tag="TID")
            IDE = cpool.tile([128, 128], f16, tag="IDE")
            RPM = cpool.tile([16, 128], f16, tag="RPM")
            for q in range(4):
                nc.sync.dma_start(Wt_["W1", q][:],
                                  w1v[q // 2][:, (q % 2) * QC:(q % 2 + 1) * QC, :])
                nc.sync.dma_start(Wt_["W2", q][:],
                                  w2v[q // 2][:, (q % 2) * QC:(q % 2 + 1) * QC, :])
                if q == 0:
                    nc.sync.dma_start(BR[:], bias_rep)
                    nc.sync.dma_start(TID[:], tid1)
                    nc.sync.dma_start(IDE[:], ident)
                    nc.sync.dma_start(RPM[:], repmat)

            def Wc(nm, c):
                return Wt_[nm, c // QC][:, c % QC, :]

            # routing on a [128,256] f32 score tile; returns (v8, i8)
            def route(s_raw, pref, mpool):
                s_choice = mpool.tile([128, E], f32, tag=f"{pref}sc")
                nc.vector.tensor_tensor(s_choice[:], s_raw[:], BR[:], op=OP.add)
                sc3 = s_choice[:].rearrange("p (g s) -> p g s", g=G)
                m1 = spool.tile([128, G], f32, tag=f"{pref}m1")
                nc.vector.reduce_max(m1[:], sc3, axis=AX.X)
                repl = mpool.tile([128, E], f32, tag=f"{pref}rp")
                nc.vector.match_replace(repl[:], m1[:], s_choice[:], -1e30)
                m2 = spool.tile([128, G], f32, tag=f"{pref}m2")
                nc.vector.reduce_max(m2[:], repl[:].rearrange("p (g s) -> p g s", g=G),
                                     axis=AX.X)
                gsum = spool.tile([128, G], f32, tag=f"{pref}gs")
                nc.gpsimd.tensor_tensor(gsum[:], m1[:], m2[:], op=OP.add)
                gs8 = spool.tile([128, 8], f32, tag=f"{pref}g8")
                nc.vector.max(gs8[:], gsum[:])
                pen = spool.tile([128, G], f32, tag=f"{pref}pn")
                nc.gpsimd.tensor_scalar(pen[:], gsum[:], gs8[:, 3:4], -1e30,
                                        op0=OP.is_lt, op1=OP.mult)
                s_mask = mpool.tile([128, E], f32, tag=f"{pref}sm")
                pen_b = pen[:].unsqueeze(2).broadcast_to([128, G, GS])
                nc.vector.tensor_tensor(
                    s_mask[:].rearrange("p (g s) -> p g s", g=G), sc3, pen_b,
                    op=OP.add)
                v8 = spool.tile([128, 8], f32, tag=f"{pref}v8")
                nc.vector.max(v8[:], s_mask[:])
                i8 = spool.tile([128, 8], u16, tag=f"{pref}i8")
                nc.vector.max_index(i8[:], v8[:], s_mask[:])
                return s_mask, gs8, v8, i8

            fvw = {w: wpool.tile([128, 16], f16, name=f"fv{w}", tag=f"fv{w}")
                   for w in range(WAVES)}

            def wave_tail(w):
                # pad cols 8..16 with -1 (never-flagged filler)
                nc.scalar.activation(fvw[w][:, 8:16], TID[:, 0:8], AF.Copy,
                                     scale=0.0, bias=-1.0)
                pt = ptpool.tile([16, 128], f16, tag="pt")
                nc.tensor.transpose(pt[:], fvw[w][:], IDE[:])
                fv32 = wpool.tile([16, 128], f32, tag="fv32")
                nc.scalar.activation(fv32[:], pt[:], AF.Copy)
                svals = wpool.tile([16, CAP // 16], f32, tag="svals")
                nf = wpool.tile([1, 1], u32, tag="nf")
                nc.gpsimd.sparse_gather(svals[:], fv32[:], num_found=nf[:])
                nc.sync.dma_start(rep_ids[w], svals[:])
                nc.sync.dma_start(rep_n[w:w + 1, :], nf[:])
                svc = wpool.tile([16, CAP // 16], f32, tag="svc")
                nc.vector.tensor_scalar(svc[:], svals[:], 0.0, float(T_CORE - 1),
                                        op0=OP.max, op1=OP.min)
                svh = wpool.tile([16, CAP // 16], f16, tag="svh")
                nc.vector.tensor_scalar(svh[:], svc[:], 0.0, None, op0=OP.add)
                ip = pipool.tile([128, CAP // 16], f32, tag="ip")
                nc.tensor.matmul(ip[:], RPM[:], svh[:], start=True, stop=True)
                ids16 = wpool.tile([128, CAP // 16], i16, tag="ids16")
                nc.vector.tensor_scalar(ids16[:], ip[:], 0.0, None, op0=OP.add)
                x1g = gpool.tile([128, N_CHUNK, CAP], f16, tag="x1g")
                x2g = gpool.tile([128, N_CHUNK, CAP], f16, tag="x2g")
                nc.gpsimd.dma_gather(x1g[:], x1t, ids16[:], CAP, CAP, H,
                                     transpose=True)
                nc.gpsimd.dma_gather(x2g[:], x2t, ids16[:], CAP, CAP, H,
                                     transpose=True)

                pr = prpool.tile([128, E], f32, tag="pr")
                pr2 = prpool.tile([128, E], f32, tag="pr2")
                for c in range(N_CHUNK):
                    nc.tensor.matmul(pr[:], x1g[:, c, :], Wc("W1", c),
                                     start=(c == 0), stop=(c == N_CHUNK - 1))
                for c in range(N_CHUNK):
                    nc.tensor.matmul(pr2[:], x1g[:, c, :], Wc("W2", c),
                                     start=(c == 0), stop=False)
                    nc.tensor.matmul(pr2[:], x2g[:, c, :], Wc("W1", c),
                                     start=False, stop=(c == N_CHUNK - 1))
                # exact sigmoid: 1/(1+exp(-x)), x = pr + pr2/CROSS
                t2n = rmpool.tile([128, E], f32, tag="rt2")
                nc.scalar.activation(t2n[:], pr2[:], AF.Copy, scale=-1.0 / CROSS)
                lgn = rmpool.tile([128, E], f32, tag="rlg")
                nc.vector.scalar_tensor_tensor(lgn[:], in0=pr[:], scalar=-1.0,
                                               in1=t2n[:], op0=OP.mult, op1=OP.add)
                ex = rmpool.tile([128, E], f32, tag="rex")
                nc.scalar.activation(ex[:], lgn[:], AF.Exp)
                ip1 = rmpool.tile([128, E], f32, tag="rip")
                nc.scalar.activation(ip1[:], ex[:], AF.Copy, bias=1.0)
                s_raw_r = rmpool.tile([128, E], f32, tag="rsr")
                rscr = rmpool.tile([128, E], f32, tag="rrs")
                nc.vector.reciprocal_approx_accurate(s_raw_r[:], ip1[:], rscr[:])
                _, _, rv8, ri8 = route(s_raw_r, "r", rmpool)
                nc.sync.dma_start(rep_v[w], rv8[:])
                nc.sync.dma_start(rep_i[w], ri8[:])

            # --- main loop ---
            for st, (toff, tsz) in enumerate(zip(ST_OFFS, ST_SIZES)):
                xt1 = xpool.tile([128, N_CHUNK, ST_MAX], f16, tag="xt1")
                seg = slice(N_CHUNK * toff, N_CHUNK * (toff + tsz))
                nc.sync.dma_start(xt1[:, :, 0:tsz],
                                  x1[:, seg].rearrange("p (c t) -> p c t", c=N_CHUNK))

                for sub in range(tsz // 128):
                    tok0 = toff + sub * 128
                    s = tok0 // 128          # global subtile 0..15
                    wv = s // SUB_W
                    ps = ppool.tile([128, E], f32, tag="ps")
                    for c in range(N_CHUNK):
                        nc.tensor.matmul(ps[:], xt1[:, c, sub * 128:(sub + 1) * 128],
                                         Wc("W1", c),
                                         start=(c == 0), stop=(c == N_CHUNK - 1))
                    s_raw = mpool.tile([128, E], f32, tag="sr")
                    nc.scalar.activation(s_raw[:], ps[:], AF.Sigmoid)
                    s_mask, gs8, v8, i8 = route(s_raw, "m", mpool)
                    nc.sync.dma_start(idx_out[tok0:tok0 + 128, :], i8[:])
                    nc.sync.dma_start(vch_out[tok0:tok0 + 128, :], v8[:])

                    # --- flag near-boundary tokens ---
                    # cnt' = sum(sign(s_mask - (v8[7]-THETA))); #{>=thr} >= 9
                    # (i.e. a 9th value within THETA of the 8th) <=> cnt' > -239.5
                    thr9n = spool.tile([128, 1], f32, tag="t9")
                    nc.scalar.activation(thr9n[:], v8[:, 7:8], AF.Copy,
                                         scale=-1.0, bias=THETA)
                    sgn = mpool.tile([128, E], f32, tag="sgn")
                    cnt = spool.tile([128, 1], f32, tag="cnt")
                    nc.scalar.activation(sgn[:], s_mask[:], AF.Sign,
                                         bias=thr9n[:, 0:1], accum_out=cnt[:])
                    ggap = spool.tile([128, 1], f32, tag="gg")
                    nc.scalar.activation(ggap[:], gs8[:, 4:5], AF.Identity,
                                         scale=-1.0, bias=gs8[:, 3:4])
                    adj = spool.tile([128, 7], f32, tag="adj")
                    nc.gpsimd.tensor_tensor(adj[:], v8[:, 0:7], v8[:, 1:8],
                                            op=OP.subtract)
                    adjmin = spool.tile([128, 1], f32, tag="am")
                    nc.vector.tensor_reduce(adjmin[:], adj[:], axis=AX.X, op=OP.min)
                    # flag = (adjmin<θ) | (ggap<θ) | (cnt>8.5), via indicator sum
                    f1 = spool.tile([128, 1], f32, tag="f1")
                    nc.gpsimd.tensor_scalar(f1[:], adjmin[:], THETA, None,
                                            op0=OP.is_lt)
                    f2 = spool.tile([128, 1], f32, tag="f2")
                    nc.gpsimd.tensor_scalar(f2[:], ggap[:], THETA, None,
                                            op0=OP.is_lt)
                    f3 = spool.tile([128, 1], f32, tag="f3")
                    nc.gpsimd.tensor_scalar(f3[:], cnt[:], -239.5, None, op0=OP.is_gt)
                    s1 = spool.tile([128, 1], f32, tag="s1")
                    nc.gpsimd.tensor_tensor(s1[:], f1[:], f2[:], op=OP.add)
                    s2 = spool.tile([128, 1], f32, tag="s2")
                    nc.gpsimd.tensor_tensor(s2[:], s1[:], f3[:], op=OP.add)
                    f12 = spool.tile([128, 1], f32, tag="f12")
                    nc.gpsimd.tensor_scalar(f12[:], s2[:], 0.5, None, op0=OP.is_gt)
                    # fv = flag * (tid+1) - 1  (tid if flagged else -1)
                    nc.scalar.activation(fvw[wv][:, (s % SUB_W):(s % SUB_W) + 1],
                                         f12[:], AF.Copy,
                                         scale=TID[:, s:s + 1], bias=-1.0)

                    if s == SUB_W - 1:
                        wave_tail(0)
                    elif s == NSUB - 1:
                        wave_tail(1)

    nc.compile()
    return nc


def _get_built():
    global _BUILT
    if _BUILT is None:
        _BUILT = _build()
    return _BUILT


def _tile_x(arr):
    # [H, T_CORE] -> [128p, N_CHUNK*T_CORE] supertile-major (c-major, token-minor)
    v = arr.reshape(N_CHUNK, 128, T_CORE)
    segs = [np.ascontiguousarray(v[:, :, o:o + s].transpose(1, 0, 2)
                                 ).reshape(128, N_CHUNK * s)
            for o, s in zip(ST_OFFS, ST_SIZES)]
    return np.ascontiguousarray(np.concatenate(segs, axis=1))


def _tile_w(arr):
    # [H, E] -> [2, 128p, HC*E] with (p,c,e): arr[(h*HC+c)*128+p, e]
    v = arr.reshape(2, HC, 128, E)
    return np.ascontiguousarray(v.transpose(0, 2, 1, 3).reshape(2, 128, HC * E))


def _prep_in_maps(hidden_states, weight, e_score_correction_bias):
    S = np.float32(CROSS)
    x = np.asarray(hidden_states, dtype=np.float32).reshape(T_FULL, H)
    x1_tok = x.astype(np.float16)                        # [T, H] token-major
    x2_tok = ((x - x1_tok.astype(np.float32)) * S).astype(np.float16)

    W = np.asarray(weight, dtype=np.float32)
    Wt = np.ascontiguousarray(W.T)                       # [H, E]
    w1 = _tile_w(Wt.astype(np.float16))
    w2 = _tile_w(((Wt - Wt.astype(np.float16).astype(np.float32)) * S
                  ).astype(np.float16))

    b = np.asarray(e_score_correction_bias, dtype=np.float32)
    bias_rep = np.ascontiguousarray(np.tile(b[None, :], (128, 1)))

    tid1 = np.ascontiguousarray(
        (np.arange(T_CORE, dtype=np.float32).reshape(NSUB, 128).T + 1.0))
    ident = np.eye(128, dtype=np.float16)
    repmat = np.ascontiguousarray(np.tile(np.eye(16, dtype=np.float16), (1, 8)))

    in_maps = []
    for c in range(N_CORES):
        sl = slice(c * T_CORE, (c + 1) * T_CORE)
        x1t_c = np.ascontiguousarray(x1_tok[sl])         # [T_CORE, H]
        x2t_c = np.ascontiguousarray(x2_tok[sl])
        in_maps.append({
            "x1": _tile_x(np.ascontiguousarray(x1t_c.T)),
            "w1": w1, "w2": w2, "bias_rep": bias_rep,
            "x1t": x1t_c, "x2t": x2t_c,
            "tid1": tid1, "ident": ident, "repmat": repmat,
        })
    return in_maps


def _merge_core(res, b):
    """Merge one core's outputs -> (idx int32 [T_CORE,8], w f32 [T_CORE,8])."""
    idx = res["idx_out"].astype(np.int32)
    v8 = res["vch_out"].astype(np.float32).copy()
    rep_i = res["rep_i"]
    rep_v = res["rep_v"]
    rep_ids = res["rep_ids"]
    rep_n = res["rep_n"]
    for w in range(WAVES):
        n = min(int(rep_n[w][0]), CAP)
        if n <= 0:
            continue
        j = np.arange(n)
        toks = rep_ids[w][j % 16, j // 16].astype(np.int64)
        idx[toks] = rep_i[w][:n].astype(np.int32)
        v8[toks] = rep_v[w][:n]
    raw = v8 - b[idx]
    wt = raw / (raw.sum(axis=-1, keepdims=True) / np.float32(SCALING))
    return idx, wt.astype(np.float32)


def kernel(hidden_states: np.ndarray, weight: np.ndarray,
           e_score_correction_bias: np.ndarray):
    in_maps = _prep_in_maps(hidden_states, weight, e_score_correction_bias)
    nc = _get_built()
    res = run_bass_kernel_spmd(nc, in_maps, list(range(N_CORES)))
    b = np.asarray(e_score_correction_bias, dtype=np.float32)
    outs = [_merge_core(r, b) for r in res.results]
    idx = np.concatenate([o[0] for o in outs], axis=0)
    w = np.concatenate([o[1] for o in outs], axis=0)
    return idx, w
